# revision 4
# baseline (speedup 1.0000x reference)
"""Trainium2 Bass kernel for nn_DecoderV1 (segment_reduce decoder).

Reference computation:
    sums[g]  = segment_sum(batch_emb, seg)        # contiguous segments of 64
    mean[g]  = sums[g] / 64
    q0[g]    = batch_emb[64*g + targets[g, 0]]
    q1[g]    = batch_emb[64*g + targets[g, 1]]
    out[g]   = concat([q0, q1, mean]) @ W.T + b   # [B, T]

Shapes (hardcoded): B=4096 graphs x 64 nodes, D=512, T=2048, fp32.

Distribution: data-parallel over graphs, 512 graphs (32768 rows of
batch_emb) per core; W/b replicated.

Device algorithm per core:
  - The segment-mean AND both query-row gathers are done on the
    TensorEngine in a single pass over batch_emb: for each 128-node tile
    (= 2 graphs), matmul with the emb tile as the STATIONARY operand
    [K=128 nodes, M=128 D-dims] (4 D-chunks) and a host-built [128, 6]
    selection matrix (one-hot(q0) x2, one-hot(q1) x2, mean-weights x2)
    as the MOVING operand.  Output lands in PSUM already transposed --
    exactly the x^T = concat([q0,q1,mean])^T layout the main GEMM needs
    for its stationary operand.
  - Main GEMM: out = x @ W.T + b with W.T pre-transposed on the host and
    resident in SBUF; x^T chunks stationary, W.T streamed, accumulated
    over the 12 k-chunks of 3D=1536 in PSUM; bias added during the
    PSUM->SBUF copy.
"""

import os
import sys
from contextlib import ExitStack

import numpy as np

sys.path.insert(0, "/opt/trn_rl_repo")

import concourse.bass as bass  # noqa: E402
import concourse.mybir as mybir  # noqa: E402
import concourse.tile as tile  # noqa: E402
from concourse import bacc  # noqa: E402
from concourse.bass_utils import run_bass_kernel_spmd  # noqa: E402

N_CORES = 8
B = 4096          # graphs total
N_PER = 64        # nodes per graph
D = 512           # embed dim
T = 2048          # target size
K3 = 3 * D        # 1536 contraction dim
B_LOC = B // N_CORES          # 512 graphs per core
N_LOC = B_LOC * N_PER         # 32768 nodes per core
NT = N_LOC // 128             # 256 node-tiles (128 nodes = 2 graphs each)
N_GROUPS = 4                  # graph groups per core (128 graphs each)
GT = NT // N_GROUPS           # 64 node-tiles per group
KC = K3 // 128                # 12 contraction chunks
F32 = mybir.dt.float32
F32R = mybir.dt.float32r

_CACHED_NC = None
LAST_RESULTS = None  # test.py reads exec_time_ns from here


def _build_nc(n_iters: int = 1):
    nc = bacc.Bacc("TRN2", target_bir_lowering=False, debug=False)

    emb = nc.dram_tensor("emb", [N_LOC, D], F32R, kind="ExternalInput")
    sel = nc.dram_tensor("sel", [N_GROUPS, 128, GT * 6], F32R, kind="ExternalInput")
    wt = nc.dram_tensor("wt", [K3, T], F32R, kind="ExternalInput")
    bias = nc.dram_tensor("bias", [128, T], F32, kind="ExternalInput")
    out = nc.dram_tensor("out", [B_LOC, T], F32, kind="ExternalOutput")

    with tile.TileContext(nc) as tc:
        with (
            tc.tile_pool(name="wt_pool", bufs=1) as wt_pool,
            tc.tile_pool(name="bias_pool", bufs=1) as bias_pool,
            tc.tile_pool(name="xt_pool", bufs=1) as xt_pool,
            tc.tile_pool(name="emb_pool", bufs=8) as emb_pool,
            tc.tile_pool(name="sel_pool", bufs=2) as sel_pool,
            tc.tile_pool(name="out_pool", bufs=2) as out_pool,
            tc.tile_pool(name="pool_psum", bufs=1, space="PSUM") as pool_psum,
            tc.tile_pool(name="gemm_psum", bufs=2, space="PSUM") as gemm_psum,
            ExitStack() as loop_ctx,
        ):
            # x^T staging: 12 chunks of [128 k, 512 graphs]
            xt = [
                xt_pool.tile([128, B_LOC], F32R, tag=f"xt{k}", name=f"xt_sb{k}")
                for k in range(KC)
            ]

            if n_iters > 1:
                # Benchmark mode: repeat the whole kernel body on-device so
                # per-iteration HW time can be extracted from wall-clock.
                loop_ctx.enter_context(tc.For_i(0, n_iters, 1))

            # W.T resident in SBUF: 12 chunks of [128 k, 2048 t] (1 MiB each)
            wt_tiles = []
            for k in range(KC):
                wtt = wt_pool.tile([128, T], F32R, tag=f"wt{k}", name=f"wt_sb{k}")
                nc.sync.dma_start(wtt[:], wt[k * 128:(k + 1) * 128, :])
                wt_tiles.append(wtt)
            bias_t = bias_pool.tile([128, T], F32, name="bias_sb")
            nc.sync.dma_start(bias_t[:], bias[:, :])

            for g in range(N_GROUPS):
                selg = sel_pool.tile([128, GT * 6], F32R, name=f"sel_sb{g}")
                nc.sync.dma_start(selg[:], sel[g])

                # 4 PSUM banks, one per D-chunk: [128 d, 64 tiles * 6 cols]
                pp = [
                    pool_psum.tile([128, GT * 6], F32, tag=f"pp{c}", name=f"pp{g}_{c}")
                    for c in range(4)
                ]

                for t in range(GT):
                    et = emb_pool.tile([128, D], F32R, tag="emb", name=f"emb_t{g}_{t}")
                    row0 = (g * GT + t) * 128
                    nc.sync.dma_start(et[:], emb[row0:row0 + 128, :])
                    for c in range(4):
                        # out[d, col] = sum_nodes emb[node, 128c+d] * sel[node, col]
                        nc.tensor.matmul(
                            pp[c][:, 6 * t:6 * t + 6],
                            et[:, 128 * c:128 * c + 128],
                            selg[:, 6 * t:6 * t + 6],
                            start=True,
                            stop=True,
                        )

                # De-interleave (q0,q0,q1,q1,m,m) tile-column groups into x^T
                # chunk tiles: role r, D-chunk c -> k-chunk r*4+c.
                for r in range(3):
                    for c in range(4):
                        src = pp[c][:].rearrange("p (t s) -> p t s", s=6)[
                            :, :, 2 * r:2 * r + 2
                        ]
                        dst = xt[r * 4 + c][:, g * 128:(g + 1) * 128].rearrange(
                            "p (t s) -> p t s", s=2
                        )
                        nc.vector.tensor_copy(dst, src)

                # Main GEMM for this group's 128 graphs:
                # out[128 graphs, T] = x[128, 1536] @ W.T[1536, T] + b
                for n in range(4):
                    ps = gemm_psum.tile([128, 512], F32, tag="gp", name=f"gp{g}_{n}")
                    for k in range(KC):
                        nc.tensor.matmul(
                            ps[:],
                            xt[k][:, g * 128:(g + 1) * 128],
                            wt_tiles[k][:, n * 512:(n + 1) * 512],
                            start=(k == 0),
                            stop=(k == KC - 1),
                        )
                    ot = out_pool.tile([128, 512], F32, tag="ot", name=f"ot{g}_{n}")
                    nc.vector.tensor_add(ot[:], ps[:], bias_t[:, n * 512:(n + 1) * 512])
                    nc.sync.dma_start(
                        out[g * 128:(g + 1) * 128, n * 512:(n + 1) * 512], ot[:]
                    )

    nc.compile()
    return nc


def _get_nc():
    global _CACHED_NC
    if _CACHED_NC is None:
        _CACHED_NC = _build_nc()
    return _CACHED_NC


def _build_sel(targets: np.ndarray) -> np.ndarray:
    """Per-core selection matrices [N_CORES, N_GROUPS, 128, GT*6] fp32.

    For group g, node-tile t (2 graphs: s=0 rows 0-63, s=1 rows 64-127),
    columns 6t..6t+5 are (q0_s0, q0_s1, q1_s0, q1_s1, mean_s0, mean_s1).
    """
    sel = np.zeros((N_CORES, N_GROUPS, 128, GT, 6), dtype=np.float32)
    sel[:, :, 0:64, :, 4] = 1.0 / N_PER
    sel[:, :, 64:128, :, 5] = 1.0 / N_PER
    # targets -> [core, group, tile, s]
    t0 = targets[:, 0].reshape(N_CORES, N_GROUPS, GT, 2).astype(np.int64)
    t1 = targets[:, 1].reshape(N_CORES, N_GROUPS, GT, 2).astype(np.int64)
    ci, gi, ti = np.meshgrid(
        np.arange(N_CORES), np.arange(N_GROUPS), np.arange(GT), indexing="ij"
    )
    for s in range(2):
        sel[ci, gi, 64 * s + t0[:, :, :, s], ti, 0 + s] = 1.0
        sel[ci, gi, 64 * s + t1[:, :, :, s], ti, 2 + s] = 1.0
    return sel.reshape(N_CORES, N_GROUPS, 128, GT * 6)


def _kernel_numpy_fallback(batch_emb, seg, targets, W, b):
    """Host fallback for unexpected segment structure (general but slow)."""
    seg = seg.astype(np.int64)
    counts = np.bincount(seg, minlength=B).astype(np.float32)
    sums = np.zeros((B, D), dtype=np.float32)
    np.add.at(sums, seg, batch_emb)
    mean = sums / counts[:, None]
    starts = np.cumsum(counts.astype(np.int64)) - counts.astype(np.int64)
    gidx = starts[:, None] + targets.astype(np.int64)
    q = batch_emb[gidx]
    x = np.concatenate([q[:, 0], q[:, 1], mean], axis=-1)
    return (x @ W.T + b).astype(np.float32)


def kernel(batch_emb, seg, targets, W, b):
    global LAST_RESULTS
    batch_emb = np.asarray(batch_emb, dtype=np.float32)
    seg = np.asarray(seg)
    targets = np.asarray(targets)
    W = np.asarray(W, dtype=np.float32)
    b = np.asarray(b, dtype=np.float32)

    expected_seg = np.repeat(np.arange(B, dtype=seg.dtype), N_PER)
    if batch_emb.shape != (B * N_PER, D) or not np.array_equal(seg, expected_seg):
        return _kernel_numpy_fallback(batch_emb, seg, targets, W, b)

    sel = _build_sel(targets)
    wt = np.ascontiguousarray(W.T)                      # [1536, 2048]
    bias = np.ascontiguousarray(np.broadcast_to(b, (128, T)))

    nc = _get_nc()
    in_maps = [
        {
            "emb": batch_emb[c * N_LOC:(c + 1) * N_LOC],
            "sel": sel[c],
            "wt": wt,
            "bias": bias,
        }
        for c in range(N_CORES)
    ]
    res = run_bass_kernel_spmd(
        nc,
        in_maps,
        list(range(N_CORES)),
        trace=bool(os.environ.get("KERNEL_TRACE")),
    )
    LAST_RESULTS = res
    return np.concatenate([res.results[c]["out"] for c in range(N_CORES)], axis=0)
